# revision 9
# baseline (speedup 1.0000x reference)
"""Trainium2 Bass kernel for nn_Cross_modal_ContrastiveLoss6.

Math: the reference loss only depends on per-class means of the two
modalities (every entry of the N x N distance matrix is determined by the
class pair), so the whole computation reduces to:

  1. raw per-class segment sums R[c,d], T[c,d]  (memory-bound: 64 MiB read)
  2. the three 128x128 class Gram matrices P1 = R R^T, P2 = T T^T, P3 = R T^T
  3. tiny 128x128 class-pair loss math with the class counts

Device strategy (8 cores, feature/d-sharded so no cross-core collective is
needed): core k takes columns [256k, 256k+256) of both modal tensors and
computes the full-N segment sums for its d-chunk with one-hot matmuls on
the PE.  fp32 matmuls run at 1/8 the bf16 rate on trn2, so the host splits
the fp32 data into bf16 (hi, lo) pairs -- exact to ~2^-17 relative, same
total DMA bytes -- and the one-hot matrix is precomputed on the host in
bf16 (0/1 exact).  Everything is packed host-side into the exact SBUF
layout ([128 partitions, free]) so each DMA is a flat contiguous copy.
The device returns the raw hi/lo segment sums; the host recombines them,
forms the three Grams, and does the count scaling + sqrt/relu/weighted
mean (<0.1% of the FLOPs) in float64.
"""

import numpy as np
import ml_dtypes

import concourse.bacc as bacc
import concourse.bass as bass
import concourse.mybir as mybir
from concourse.bass_utils import run_bass_kernel_spmd
from concourse.tile import TileContext

N = 4096
D = 2048
C = 128
MARGIN = 0.5
NCORES = 8
DCHUNK = D // NCORES          # 256 feature columns per core
P = 128                       # partitions / sample-block size
NB = N // P                   # 32 sample blocks
CHUNK_B = 4                   # sample blocks per x-DMA (512 KiB)
NCHUNK = NB // CHUNK_B        # 8 DMA chunks per modal
OH_CHUNK_B = 8                # sample blocks per one-hot DMA (256 KiB)

F32 = mybir.dt.float32
BF16 = mybir.dt.bfloat16
NPBF16 = ml_dtypes.bfloat16

_PROGRAM = None


def _build_program() -> bass.Bass:
    """Raw-bass program (no TileContext): 4 engine streams, 5 semaphores.

    sync ring:   oh chunks + x1 chunks (interleaved) -> out DMA at the end
    scalar ring: x2 chunks
    tensor:      2 accumulation groups of 32 bf16 [128x128]x[128x512] matmuls
    vector:      2 PSUM->SBUF copies of the finished sums
    """
    nc = bass.Bass()

    # All inputs are packed host-side as [128 partitions, free] where
    # partition p of sample-block b is sample b*128+p.
    oh_in = nc.declare_dram_parameter("oh", [P, NB * C], BF16, isOutput=False)
    x1_in = nc.declare_dram_parameter("x1", [P, NB * 512], BF16, isOutput=False)
    x2_in = nc.declare_dram_parameter("x2", [P, NB * 512], BF16, isOutput=False)
    # sums[:, 0:512]  = R hi|lo partial sums   [class, 2*256]
    # sums[:, 512:1024] = T hi|lo partial sums
    sums_out = nc.declare_dram_parameter("sums", [P, 1024], F32, isOutput=True)

    XW = CHUNK_B * 512      # x columns per DMA chunk
    OHW = OH_CHUNK_B * C    # oh columns per DMA chunk
    NOH = NB // OH_CHUNK_B  # one-hot DMA chunks

    # sync-ring issue order: oh0, x1_0, oh1, x1_1, oh2, x1_2, oh3, x1_3,
    # x1_4 .. x1_7.  dma1 counts 16 per completed DMA, in issue order.
    def x1_done(j):  # dma1 value once x1 chunk j (and all before it) landed
        idx = 2 * j + 1 if j < NOH else NOH + j
        return 16 * (idx + 1)

    with (
        nc.sbuf_tensor([P, NB * C], BF16) as oh_t,
        nc.sbuf_tensor([P, NB * 512], BF16) as x1_t,
        nc.sbuf_tensor([P, NB * 512], BF16) as x2_t,
        nc.sbuf_tensor([P, 1024], F32) as out_t,
        nc.psum_tensor([P, 512], F32) as psum_r,
        nc.psum_tensor([P, 512], F32) as psum_t,
        nc.semaphore("dma1") as dma1,
        nc.semaphore("dma2") as dma2,
        nc.semaphore("pe_done") as pe_done,
        nc.semaphore("vec_done") as vec_done,
        nc.semaphore("dma_out") as dma_out,
        nc.Block() as block,
    ):

        @block.sync
        def _(sync: bass.BassEngine):
            for j in range(NCHUNK):
                if j < NOH:
                    sync.dma_start(
                        out=oh_t[:, j * OHW : (j + 1) * OHW],
                        in_=oh_in[:, j * OHW : (j + 1) * OHW],
                    ).then_inc(dma1, 16)
                sync.dma_start(
                    out=x1_t[:, j * XW : (j + 1) * XW],
                    in_=x1_in[:, j * XW : (j + 1) * XW],
                ).then_inc(dma1, 16)
            sync.wait_ge(vec_done, 2)
            sync.dma_start(out=sums_out[:], in_=out_t[:]).then_inc(dma_out, 16)
            sync.wait_ge(dma_out, 16)

        @block.scalar
        def _(scalar: bass.BassEngine):
            for j in range(NCHUNK):
                scalar.dma_start(
                    out=x2_t[:, j * XW : (j + 1) * XW],
                    in_=x2_in[:, j * XW : (j + 1) * XW],
                ).then_inc(dma2, 16)

        @block.tensor
        def _(tensor: bass.BassEngine):
            for j in range(NCHUNK):
                tensor.wait_ge(dma1, x1_done(j))
                for bb in range(CHUNK_B):
                    b = j * CHUNK_B + bb
                    mm = nc.tensor.matmul(
                        psum_r[:],
                        oh_t[:, b * C : (b + 1) * C],
                        x1_t[:, b * 512 : (b + 1) * 512],
                        start=(b == 0),
                        stop=(b == NB - 1),
                    )
                    if b == NB - 1:
                        mm.then_inc(pe_done, 1)
                tensor.wait_ge(dma2, 16 * (j + 1))
                for bb in range(CHUNK_B):
                    b = j * CHUNK_B + bb
                    mm = nc.tensor.matmul(
                        psum_t[:],
                        oh_t[:, b * C : (b + 1) * C],
                        x2_t[:, b * 512 : (b + 1) * 512],
                        start=(b == 0),
                        stop=(b == NB - 1),
                    )
                    if b == NB - 1:
                        mm.then_inc(pe_done, 1)

        @block.vector
        def _(vector: bass.BassEngine):
            vector.wait_ge(pe_done, 2)
            nc.vector.tensor_copy(out_t[:, 0:512], psum_r[:]).then_inc(vec_done, 1)
            nc.vector.tensor_copy(out_t[:, 512:1024], psum_t[:]).then_inc(vec_done, 1)

    return nc


def _get_program() -> bass.Bass:
    global _PROGRAM
    if _PROGRAM is None:
        _PROGRAM = _build_program()
    return _PROGRAM


def _pack_blocks(x):
    """[4096, W] -> [128, NB*W] with partition p, block b at cols [b*W,(b+1)*W)."""
    W = x.shape[1]
    return np.ascontiguousarray(
        x.reshape(NB, P, W).transpose(1, 0, 2).reshape(P, NB * W)
    )


def _make_in_maps(modal1, modal2, targets):
    x1 = np.asarray(modal1, dtype=np.float32)
    x2 = np.asarray(modal2, dtype=np.float32)
    targets = np.asarray(targets)

    # bf16 hi/lo split (exact to ~2^-17 relative)
    def hilo(x):
        hi = x.astype(NPBF16)
        lo = (x - hi.astype(np.float32)).astype(NPBF16)
        return hi, lo

    x1_hi, x1_lo = hilo(x1)
    x2_hi, x2_lo = hilo(x2)

    oh = (targets[:, None] == np.arange(C)[None, :]).astype(NPBF16)  # [N, C]
    oh_packed = _pack_blocks(oh)

    in_maps = []
    for k in range(NCORES):
        sl = slice(k * DCHUNK, (k + 1) * DCHUNK)

        def pack_modal(hi, lo):
            # [4096, 512] = hi | lo for this core's d-chunk
            hl = np.concatenate([hi[:, sl], lo[:, sl]], axis=1)
            return _pack_blocks(hl)

        in_maps.append(
            {
                "oh": oh_packed,
                "x1": pack_modal(x1_hi, x1_lo),
                "x2": pack_modal(x2_hi, x2_lo),
            }
        )
    return in_maps


def _finish_on_host(sums_list, targets):
    """Recombine hi/lo sums, form class Grams, and do the class-pair loss."""
    P1 = np.zeros((C, C), np.float64)
    P2 = np.zeros((C, C), np.float64)
    P3 = np.zeros((C, C), np.float64)
    for s in sums_list:
        s = np.asarray(s, np.float64)
        R = s[:, 0:256] + s[:, 256:512]      # [class, d-chunk]
        T = s[:, 512:768] + s[:, 768:1024]
        P1 += R @ R.T
        P2 += T @ T.T
        P3 += R @ T.T

    n = np.bincount(targets, minlength=C).astype(np.float64)
    u = 1.0 / np.maximum(n, 1.0)

    S_CC = P1 + P2 + P3 + P3.T  # (R+T)(R+T)^T
    uu = np.outer(u, u)
    A1 = 0.5 * uu * (P1 + P3)    # meanR . ctr
    A2 = 0.5 * uu * (P2 + P3.T)  # meanT . ctr
    nR = u * u * np.diag(P1)
    nT = u * u * np.diag(P2)
    nCtr = 0.25 * u * u * np.diag(S_CC)

    W = np.outer(n, n)
    eye = np.eye(C)
    total = 0.0
    for A, nrm in ((A1, nR), (A2, nT)):
        sq = np.maximum(nrm[:, None] + nCtr[None, :] - 2.0 * A, 1e-12)
        d = np.sqrt(sq)
        dd = np.sqrt(d + 1e-10)
        term = eye * sq + (1.0 - eye) * np.maximum(MARGIN - dd, 0.0) ** 2
        total += (W * term).sum() / (float(N) * float(N))
    return np.asarray(total, dtype=np.float32)


def kernel(modal1_inputs, modal2_inputs, targets):
    nc = _get_program()
    in_maps = _make_in_maps(modal1_inputs, modal2_inputs, targets)
    res = run_bass_kernel_spmd(nc, in_maps, list(range(NCORES)))
    sums_list = [res.results[k]["sums"] for k in range(NCORES)]
    return _finish_on_host(sums_list, np.asarray(targets))


# revision 11
# speedup vs baseline: 1.0298x; 1.0298x over previous
"""Trainium2 Bass kernel for nn_Cross_modal_ContrastiveLoss6.

Math: the reference loss only depends on per-class means of the two
modalities (every entry of the N x N distance matrix is determined by the
class pair), so the whole computation reduces to:

  1. raw per-class segment sums R[c,d], T[c,d]  (memory-bound: 64 MiB read)
  2. the three 128x128 class Gram matrices P1 = R R^T, P2 = T T^T, P3 = R T^T
  3. tiny 128x128 class-pair loss math with the class counts

Device strategy (8 cores, feature/d-sharded so no cross-core collective is
needed): core k takes columns [256k, 256k+256) of both modal tensors and
computes the full-N segment sums for its d-chunk with one-hot matmuls on
the PE.  fp32 matmuls run at 1/8 the bf16 rate on trn2, so the host splits
the fp32 data into bf16 (hi, lo) pairs -- exact to ~2^-17 relative, same
total DMA bytes -- and the one-hot matrix is precomputed on the host in
bf16 (0/1 exact).  Everything is packed host-side into the exact SBUF
layout ([128 partitions, free]) so each DMA is a flat contiguous copy.
The device returns the raw hi/lo segment sums; the host recombines them,
forms the three Grams, and does the count scaling + sqrt/relu/weighted
mean (<0.1% of the FLOPs) in float64.
"""

import numpy as np
import ml_dtypes

import concourse.bacc as bacc
import concourse.bass as bass
import concourse.mybir as mybir
from concourse.bass_utils import run_bass_kernel_spmd
from concourse.tile import TileContext

N = 4096
D = 2048
C = 128
MARGIN = 0.5
NCORES = 8
DCHUNK = D // NCORES          # 256 feature columns per core
P = 128                       # partitions / sample-block size
NB = N // P                   # 32 sample blocks
CHUNK_B = 4                   # sample blocks per x-DMA (512 KiB)
NCHUNK = NB // CHUNK_B        # 8 DMA chunks per modal
OH_CHUNK_B = 8                # sample blocks per one-hot DMA (256 KiB)

F32 = mybir.dt.float32
BF16 = mybir.dt.bfloat16
NPBF16 = ml_dtypes.bfloat16

_PROGRAM = None


def _build_program() -> bass.Bass:
    """Raw-bass program (no TileContext): 4 engine streams, 5 semaphores.

    sync ring:   oh chunks + x1 chunks (interleaved) -> out DMA at the end
    scalar ring: x2 chunks
    tensor:      2 accumulation groups of 32 bf16 [128x128]x[128x512] matmuls
    vector:      2 PSUM->SBUF copies of the finished sums
    """
    nc = bass.Bass()

    # All inputs are packed host-side as [128 partitions, free] where
    # partition p of sample-block b is sample b*128+p.
    oh_in = nc.declare_dram_parameter("oh", [P, NB * C], BF16, isOutput=False)
    x1_in = nc.declare_dram_parameter("x1", [P, NB * 512], BF16, isOutput=False)
    x2_in = nc.declare_dram_parameter("x2", [P, NB * 512], BF16, isOutput=False)
    # sums[:, 0:512]  = R hi|lo partial sums   [class, 2*256]
    # sums[:, 512:1024] = T hi|lo partial sums
    sums_out = nc.declare_dram_parameter("sums", [P, 1024], F32, isOutput=True)

    XW = CHUNK_B * 512      # x columns per DMA chunk
    OHW = OH_CHUNK_B * C    # oh columns per DMA chunk
    NOH = NB // OH_CHUNK_B  # one-hot DMA chunks

    # sync-ring issue order: oh0, x1_0, oh1, x1_1, oh2, x1_2, oh3, x1_3,
    # x1_4 .. x1_7.  dma1 counts 16 per completed DMA, in issue order.
    def x1_done(j):  # dma1 value once x1 chunk j (and all before it) landed
        idx = 2 * j + 1 if j < NOH else NOH + j
        return 16 * (idx + 1)

    with (
        nc.sbuf_tensor([P, NB * C], BF16) as oh_t,
        nc.sbuf_tensor([P, NB * 512], BF16) as x1_t,
        nc.sbuf_tensor([P, NB * 512], BF16) as x2_t,
        nc.sbuf_tensor([P, 1024], F32) as out_t,
        nc.psum_tensor([P, 512], F32) as psum_r,
        nc.psum_tensor([P, 512], F32) as psum_t,
        nc.semaphore("dma1") as dma1,
        nc.semaphore("dma2") as dma2,
        nc.semaphore("pe_done") as pe_done,
        nc.semaphore("vec_done") as vec_done,
        nc.semaphore("dma_out") as dma_out,
        nc.Block() as block,
    ):

        @block.sync
        def _(sync: bass.BassEngine):
            for j in range(NCHUNK):
                if j < NOH:
                    sync.dma_start(
                        out=oh_t[:, j * OHW : (j + 1) * OHW],
                        in_=oh_in[:, j * OHW : (j + 1) * OHW],
                    ).then_inc(dma1, 16)
                sync.dma_start(
                    out=x1_t[:, j * XW : (j + 1) * XW],
                    in_=x1_in[:, j * XW : (j + 1) * XW],
                ).then_inc(dma1, 16)
            sync.wait_ge(vec_done, 1)
            sync.dma_start(out=sums_out[:], in_=out_t[:]).then_inc(dma_out, 16)
            sync.wait_ge(dma_out, 16)

        @block.scalar
        def _(scalar: bass.BassEngine):
            for j in range(NCHUNK):
                scalar.dma_start(
                    out=x2_t[:, j * XW : (j + 1) * XW],
                    in_=x2_in[:, j * XW : (j + 1) * XW],
                ).then_inc(dma2, 16)

        @block.tensor
        def _(tensor: bass.BassEngine):
            for j in range(NCHUNK):
                tensor.wait_ge(dma1, x1_done(j))
                for bb in range(CHUNK_B):
                    b = j * CHUNK_B + bb
                    nc.tensor.matmul(
                        psum_r[:],
                        oh_t[:, b * C : (b + 1) * C],
                        x1_t[:, b * 512 : (b + 1) * 512],
                        start=(b == 0),
                        stop=(b == NB - 1),
                    )
                tensor.wait_ge(dma2, 16 * (j + 1))
                for bb in range(CHUNK_B):
                    b = j * CHUNK_B + bb
                    nc.tensor.matmul(
                        psum_t[:],
                        oh_t[:, b * C : (b + 1) * C],
                        x2_t[:, b * 512 : (b + 1) * 512],
                        start=(b == 0),
                        stop=(b == NB - 1),
                    )
            # drain makes sure the last matmuls' PSUM writes have landed
            # before the DVE reads them.
            tensor.drain().then_inc(pe_done, 1)

        @block.vector
        def _(vector: bass.BassEngine):
            vector.wait_ge(pe_done, 1)
            nc.vector.tensor_copy(out_t[:, 0:512], psum_r[:])
            nc.vector.tensor_copy(out_t[:, 512:1024], psum_t[:])
            vector.drain().then_inc(vec_done, 1)

    return nc


def _get_program() -> bass.Bass:
    global _PROGRAM
    if _PROGRAM is None:
        _PROGRAM = _build_program()
    return _PROGRAM


def _pack_blocks(x):
    """[4096, W] -> [128, NB*W] with partition p, block b at cols [b*W,(b+1)*W)."""
    W = x.shape[1]
    return np.ascontiguousarray(
        x.reshape(NB, P, W).transpose(1, 0, 2).reshape(P, NB * W)
    )


def _make_in_maps(modal1, modal2, targets):
    x1 = np.asarray(modal1, dtype=np.float32)
    x2 = np.asarray(modal2, dtype=np.float32)
    targets = np.asarray(targets)

    # bf16 hi/lo split (exact to ~2^-17 relative)
    def hilo(x):
        hi = x.astype(NPBF16)
        lo = (x - hi.astype(np.float32)).astype(NPBF16)
        return hi, lo

    x1_hi, x1_lo = hilo(x1)
    x2_hi, x2_lo = hilo(x2)

    oh = (targets[:, None] == np.arange(C)[None, :]).astype(NPBF16)  # [N, C]
    oh_packed = _pack_blocks(oh)

    in_maps = []
    for k in range(NCORES):
        sl = slice(k * DCHUNK, (k + 1) * DCHUNK)

        def pack_modal(hi, lo):
            # [4096, 512] = hi | lo for this core's d-chunk
            hl = np.concatenate([hi[:, sl], lo[:, sl]], axis=1)
            return _pack_blocks(hl)

        in_maps.append(
            {
                "oh": oh_packed,
                "x1": pack_modal(x1_hi, x1_lo),
                "x2": pack_modal(x2_hi, x2_lo),
            }
        )
    return in_maps


def _finish_on_host(sums_list, targets):
    """Recombine hi/lo sums, form class Grams, and do the class-pair loss."""
    P1 = np.zeros((C, C), np.float64)
    P2 = np.zeros((C, C), np.float64)
    P3 = np.zeros((C, C), np.float64)
    for s in sums_list:
        s = np.asarray(s, np.float64)
        R = s[:, 0:256] + s[:, 256:512]      # [class, d-chunk]
        T = s[:, 512:768] + s[:, 768:1024]
        P1 += R @ R.T
        P2 += T @ T.T
        P3 += R @ T.T

    n = np.bincount(targets, minlength=C).astype(np.float64)
    u = 1.0 / np.maximum(n, 1.0)

    S_CC = P1 + P2 + P3 + P3.T  # (R+T)(R+T)^T
    uu = np.outer(u, u)
    A1 = 0.5 * uu * (P1 + P3)    # meanR . ctr
    A2 = 0.5 * uu * (P2 + P3.T)  # meanT . ctr
    nR = u * u * np.diag(P1)
    nT = u * u * np.diag(P2)
    nCtr = 0.25 * u * u * np.diag(S_CC)

    W = np.outer(n, n)
    eye = np.eye(C)
    total = 0.0
    for A, nrm in ((A1, nR), (A2, nT)):
        sq = np.maximum(nrm[:, None] + nCtr[None, :] - 2.0 * A, 1e-12)
        d = np.sqrt(sq)
        dd = np.sqrt(d + 1e-10)
        term = eye * sq + (1.0 - eye) * np.maximum(MARGIN - dd, 0.0) ** 2
        total += (W * term).sum() / (float(N) * float(N))
    return np.asarray(total, dtype=np.float32)


def kernel(modal1_inputs, modal2_inputs, targets):
    nc = _get_program()
    in_maps = _make_in_maps(modal1_inputs, modal2_inputs, targets)
    res = run_bass_kernel_spmd(nc, in_maps, list(range(NCORES)))
    sums_list = [res.results[k]["sums"] for k in range(NCORES)]
    return _finish_on_host(sums_list, np.asarray(targets))
